# revision 8
# baseline (speedup 1.0000x reference)
"""MGU (minimal gated unit) Bass kernel for Trainium2, 8-core SPMD.

Problem: B=128, T=512, D=U=512 fp32.
    xf = x @ Wf + bf ; xh = x @ Wh + bh            (parallel over B,T)
    scan over t: f = sigmoid(xf_t + h @ Uf)
                 S = tanh(xh_t + (f*h) @ Uh)
                 h = (1-f)*h + f*S
Output: final h [B, U].

Sharding: data-parallel over B (16 rows/core), weights replicated.

Layout ("T-layout"): U (or D) stays on the partition axis, batch on the
free axis, so the sequential recurrence needs no per-step transposes:
  - h/f/S/g tiles: [128p, kt*16b] = [128, 64]   (kt = U/128 = 4)
  - per-step matmul zT[m] = sum_k Uf[k,m].T @ hT[k] -> [128, 4*16] PSUM

Perf structure (v2):
  - Recurrent weights Uf/Uh in fp8e3m4 scaled x128: halves LDWEIGHTS
    bandwidth, which dominates PE time (16 [128,128] block loads per
    gate, N=16 moving). Scale is folded into the bf16 projection
    weights / biases and removed by the free ACT scale=1/128.
  - xf_t/xh_t are seeded into the PSUM accumulator by an identity-weight
    matmul (sets has_written); the zh seed and next-step zf seed are
    emitted into the PE gaps left by the serial ACT/DVE chain.
  - f/g/S/t2/t3/h all bf16 -> DVE 2x mode on the chain ops.
  - phase-1 projection matmuls (bf16) interleave into the scan's PE gaps.
  - t2 = h - g runs on the idle GpSimd engine, off the critical chain.
"""

import os
import numpy as np
import ml_dtypes

import concourse.bass as bass
import concourse.bacc as bacc
import concourse.mybir as mybir
from concourse import tile
from concourse.bass_utils import run_bass_kernel_spmd

B, T, D, U = 128, 512, 512, 512
NCORES = 8
BC = B // NCORES          # batch rows per core = 16
KT = D // 128             # 4 contraction tiles
MT = U // 128             # 4 output tiles
CHUNK = 32                # phase-1 time-chunk; N = CHUNK*BC = 512 per matmul
GW = MT * BC              # scan tile width = 64

SCALE = 128.0             # fp8e3m4 weight pre-scale
INV = 1.0 / SCALE

BF16 = mybir.dt.bfloat16
F8E3 = mybir.dt.float8e3
F32 = mybir.dt.float32
NPBF16 = ml_dtypes.bfloat16
NPF8 = ml_dtypes.float8_e3m4
AF = mybir.ActivationFunctionType
ALU = mybir.AluOpType

_CACHE = {}
LAST_RESULTS = None  # test harness reads exec_time_ns / profile from here


def _build(t_steps: int):
    nc = bacc.Bacc("TRN2", target_bir_lowering=False, debug=False)
    nchunk = (t_steps + CHUNK - 1) // CHUNK

    x_d = nc.dram_tensor("xT", [KT, 128, T * BC], BF16, kind="ExternalInput")
    wf_d = nc.dram_tensor("WfT", [128, KT * U], BF16, kind="ExternalInput")
    wh_d = nc.dram_tensor("WhT", [128, KT * U], BF16, kind="ExternalInput")
    uf_d = nc.dram_tensor("UfT", [128, KT * U], F8E3, kind="ExternalInput")
    uh_d = nc.dram_tensor("UhT", [128, KT * U], F8E3, kind="ExternalInput")
    bf_d = nc.dram_tensor("bfT", [128, MT], F32, kind="ExternalInput")
    bh_d = nc.dram_tensor("bhT", [128, MT], F32, kind="ExternalInput")
    eye_d = nc.dram_tensor("eye", [128, 128], F8E3, kind="ExternalInput")
    out_d = nc.dram_tensor("hT_out", [128, KT * BC], F32, kind="ExternalOutput")

    with tile.TileContext(nc) as tc:
        with (
            tc.tile_pool(name="const", bufs=1) as cpool,
            tc.tile_pool(name="xchunk", bufs=3) as xpool,
            tc.tile_pool(name="proj", bufs=16) as projpool,
            tc.tile_pool(name="work", bufs=4) as wpool,
            tc.tile_pool(name="spsum", bufs=4, space="PSUM") as spsum,
            tc.tile_pool(name="ppsum", bufs=2, space="PSUM") as ppsum,
        ):
            # ---- resident tensors ----
            wf_sb = cpool.tile([128, KT * U], BF16, tag="wf")
            wh_sb = cpool.tile([128, KT * U], BF16, tag="wh")
            uf_sb = cpool.tile([128, KT * U], F8E3, tag="uf")
            uh_sb = cpool.tile([128, KT * U], F8E3, tag="uh")
            bf_sb = cpool.tile([128, MT], F32, tag="bf")
            bh_sb = cpool.tile([128, MT], F32, tag="bh")
            eye_sb = cpool.tile([128, 128], F8E3, tag="eye")

            nc.sync.dma_start(wf_sb[:], wf_d[:])
            nc.sync.dma_start(wh_sb[:], wh_d[:])
            nc.sync.dma_start(uf_sb[:], uf_d[:])
            nc.sync.dma_start(uh_sb[:], uh_d[:])
            nc.sync.dma_start(bf_sb[:], bf_d[:])
            nc.sync.dma_start(bh_sb[:], bh_d[:])
            nc.sync.dma_start(eye_sb[:], eye_d[:])

            # per-chunk projection tiles (bf16): free = (t_local, m, b)
            xf_c = [None] * nchunk
            xh_c = [None] * nchunk
            xc_c = [None] * nchunk

            def emit_chunk_dma(c):
                xc = xpool.tile([128, KT * CHUNK * BC], BF16, tag="xc")
                for k in range(KT):
                    nc.sync.dma_start(
                        xc[:, k * CHUNK * BC:(k + 1) * CHUNK * BC],
                        x_d[k, :, c * CHUNK * BC:(c + 1) * CHUNK * BC],
                    )
                xc_c[c] = xc
                xf_c[c] = projpool.tile([128, CHUNK * GW], BF16, tag="xfc", name=f"xfc{c}")
                xh_c[c] = projpool.tile([128, CHUNK * GW], BF16, tag="xhc", name=f"xhc{c}")

            def proj_group_items(c, gi):
                """One (gate, m) projection group of chunk c as a list of
                closures: 4 matmuls + ACT copy, to be fed into scan PE gaps."""
                gate, m = divmod(gi, MT)
                w_sb, b_sb, dst = ((wf_sb, bf_sb, xf_c[c]), (wh_sb, bh_sb, xh_c[c]))[gate]
                xc = xc_c[c]
                state = {}

                def mk_mm(k):
                    def emit():
                        if k == 0:
                            state["ps"] = ppsum.tile(
                                [128, CHUNK * BC], F32, tag="pp", name=f"pp{c}_{gi}"
                            )
                        nc.tensor.matmul(
                            state["ps"][:],
                            w_sb[:, k * U + m * 128: k * U + (m + 1) * 128],
                            xc[:, k * CHUNK * BC:(k + 1) * CHUNK * BC],
                            start=(k == 0), stop=(k == KT - 1),
                            skip_group_check=True,
                        )
                    return emit

                def mk_act():
                    def emit():
                        dv = dst[:].rearrange("p (t m b) -> p t m b", t=CHUNK, m=MT, b=BC)
                        nc.scalar.activation(
                            dv[:, :, m, :],
                            state["ps"][:].rearrange("p (t b) -> p t b", t=CHUNK, b=BC),
                            AF.Identity,
                            bias=b_sb[:, m:m + 1],
                        )
                    return emit

                return [mk_mm(k) for k in range(KT)] + [mk_act()]

            proj_q = []

            def pop_proj(n):
                for _ in range(n):
                    if proj_q:
                        proj_q.pop(0)()

            # prologue: first two chunks fully
            for c in range(min(2, nchunk)):
                emit_chunk_dma(c)
                for gi in range(2 * MT):
                    for item in proj_group_items(c, gi):
                        item()

            # ---- the sequential scan, with projection work interleaved ----
            h = wpool.tile([128, GW], BF16, tag="h")
            nc.vector.memset(h[:], 0.0)

            def gate_accum(z, u_sb, rhs):
                for m in range(MT):
                    for k in range(KT):
                        nc.tensor.matmul(
                            z[:, m * BC:(m + 1) * BC],
                            u_sb[:, k * U + m * 128: k * U + (m + 1) * 128],
                            rhs[:, k * BC:(k + 1) * BC],
                            start=False, stop=(m == MT - 1 and k == KT - 1),
                            skip_group_check=True,
                        )

            def seed(z, xsrc):
                nc.tensor.matmul(z[:], eye_sb[:], xsrc, start=True, stop=False,
                                 skip_group_check=True)

            # z tiles are allocated full-bank ([128, 512] fp32) so the 4-deep
            # rotation maps to 4 distinct PSUM banks: the WAR on a bank then
            # trails by 2 steps and the seed matmuls can run inside the
            # ACT/DVE gaps instead of stalling on the current sigmoid/tanh.
            ZB = 512

            # pre-seed zf for step 0
            zf = spsum.tile([128, ZB], F32, tag="z")
            seed(zf[:, 0:GW], xf_c[0][:, 0:GW])

            for t in range(t_steps):
                c, tl = divmod(t, CHUNK)
                # interleave next-next chunk's projection work into PE gaps
                nxt = c + 2
                if nxt < nchunk:
                    if tl == 0:
                        emit_chunk_dma(nxt)
                        for gi in range(2 * MT):
                            proj_q.extend(proj_group_items(nxt, gi))

                gate_accum(zf[:, 0:GW], uf_sb, h)

                # seed zh now: PE fills the sigmoid/mult gap with it
                zh = spsum.tile([128, ZB], F32, tag="z")
                seed(zh[:, 0:GW], xh_c[c][:, tl * GW:(tl + 1) * GW])
                pop_proj(1)

                f = wpool.tile([128, GW], BF16, tag="f")
                nc.scalar.activation(f[:], zf[:, 0:GW], AF.Sigmoid, scale=INV)
                g = wpool.tile([128, GW], BF16, tag="g")
                nc.vector.tensor_tensor(g[:], f[:], h[:], ALU.mult)
                t2 = wpool.tile([128, GW], BF16, tag="t2")
                nc.gpsimd.tensor_tensor(t2[:], h[:], g[:], ALU.subtract)

                gate_accum(zh[:, 0:GW], uh_sb, g)

                # pre-seed next step's zf: fills the tanh/update gap
                if t + 1 < t_steps:
                    c1, tl1 = divmod(t + 1, CHUNK)
                    zf = spsum.tile([128, ZB], F32, tag="z")
                    seed(zf[:, 0:GW], xf_c[c1][:, tl1 * GW:(tl1 + 1) * GW])
                pop_proj(1)

                s = wpool.tile([128, GW], BF16, tag="s")
                nc.scalar.activation(s[:], zh[:, 0:GW], AF.Tanh, scale=INV)

                # h' = t2 + f*S
                t3 = wpool.tile([128, GW], BF16, tag="t3")
                nc.vector.tensor_tensor(t3[:], f[:], s[:], ALU.mult)
                last = (t == t_steps - 1)
                hn = wpool.tile([128, GW], F32 if last else BF16, tag="hout" if last else "h")
                nc.vector.tensor_tensor(hn[:], t2[:], t3[:], ALU.add)
                h = hn

            nc.sync.dma_start(out_d[:], h[:])

    nc.compile()
    return nc


def _prep_weight_t(w, dtype):
    # [D, U] fp32 -> [128, KT*U] with [:, k*U+m] = w[k*128+p, m]
    return np.ascontiguousarray(
        w.reshape(KT, 128, U).transpose(1, 0, 2).reshape(128, KT * U)
    ).astype(dtype)


def kernel(x, Wf, Uf, bf, Wh, Uh, bh):
    global LAST_RESULTS
    x = np.asarray(x, dtype=np.float32)
    Wf = np.asarray(Wf, dtype=np.float32)
    Uf = np.asarray(Uf, dtype=np.float32)
    Wh = np.asarray(Wh, dtype=np.float32)
    Uh = np.asarray(Uh, dtype=np.float32)
    bf = np.asarray(bf, dtype=np.float32)
    bh = np.asarray(bh, dtype=np.float32)

    t_steps = int(os.environ.get("BASS_MGU_T", T))
    if t_steps not in _CACHE:
        _CACHE[t_steps] = _build(t_steps)
    nc = _CACHE[t_steps]

    wf_t = _prep_weight_t(Wf * SCALE, NPBF16)
    wh_t = _prep_weight_t(Wh * SCALE, NPBF16)
    uf_t = _prep_weight_t(np.clip(Uf * SCALE, -15.5, 15.5), NPF8)
    uh_t = _prep_weight_t(np.clip(Uh * SCALE, -15.5, 15.5), NPF8)
    bf_t = np.ascontiguousarray((bf * SCALE).reshape(MT, 128).T).astype(np.float32)
    bh_t = np.ascontiguousarray((bh * SCALE).reshape(MT, 128).T).astype(np.float32)
    eye = np.eye(128, dtype=np.float32).astype(NPF8)

    in_maps = []
    for ci in range(NCORES):
        xc = x[ci * BC:(ci + 1) * BC]                       # [BC, T, D]
        xt = xc.transpose(2, 1, 0)                          # [D, T, BC]
        xt = np.ascontiguousarray(xt.reshape(KT, 128, T * BC)).astype(NPBF16)
        in_maps.append({
            "xT": xt, "WfT": wf_t, "WhT": wh_t, "UfT": uf_t, "UhT": uh_t,
            "bfT": bf_t, "bhT": bh_t, "eye": eye,
        })

    trace = bool(int(os.environ.get("BASS_MGU_TRACE", "0")))
    kw = {}
    if trace and os.environ.get("BASS_TRACE_DIR"):
        kw["tmpdir"] = os.environ["BASS_TRACE_DIR"]
    res = run_bass_kernel_spmd(nc, in_maps, list(range(NCORES)), trace=trace, **kw)
    LAST_RESULTS = res

    out = np.empty((B, U), dtype=np.float32)
    for ci in range(NCORES):
        ho = np.asarray(res.results[ci]["hT_out"])          # [128, KT*BC]
        out[ci * BC:(ci + 1) * BC] = (
            ho.reshape(128, KT, BC).transpose(2, 1, 0).reshape(BC, U)
        )
    return out


# revision 10
# speedup vs baseline: 1.1172x; 1.1172x over previous
"""MGU (minimal gated unit) Bass kernel for Trainium2, 8-core SPMD.

Problem: B=128, T=512, D=U=512 fp32.
    xf = x @ Wf + bf ; xh = x @ Wh + bh            (parallel over B,T)
    scan over t: f = sigmoid(xf_t + h @ Uf)
                 S = tanh(xh_t + (f*h) @ Uh)
                 h = (1-f)*h + f*S
Output: final h [B, U].

Sharding: data-parallel over B (16 rows/core), weights replicated.

Layout ("T-layout"): U (or D) stays on the partition axis, batch on the
free axis, so the sequential recurrence needs no per-step transposes:
  - h/f/S/g tiles: [128p, kt*16b] = [128, 64]   (kt = U/128 = 4)
  - per-step matmul zT[m] = sum_k Uf[k,m].T @ hT[k] -> [128, 4*16] PSUM

Perf structure (v2):
  - Recurrent weights Uf/Uh in fp8e3m4 scaled x128: halves LDWEIGHTS
    bandwidth, which dominates PE time (16 [128,128] block loads per
    gate, N=16 moving). Scale is folded into the bf16 projection
    weights / biases and removed by the free ACT scale=1/128.
  - xf_t/xh_t are seeded into the PSUM accumulator by an identity-weight
    matmul (sets has_written); the zh seed and next-step zf seed are
    emitted into the PE gaps left by the serial ACT/DVE chain.
  - f/g/S/t2/t3/h all bf16 -> DVE 2x mode on the chain ops.
  - phase-1 projection matmuls (bf16) interleave into the scan's PE gaps.
  - t2 = h - g runs on the idle GpSimd engine, off the critical chain.
"""

import os
import numpy as np
import ml_dtypes

import concourse.bass as bass
import concourse.bacc as bacc
import concourse.mybir as mybir
from concourse import tile
from concourse.bass_utils import run_bass_kernel_spmd

B, T, D, U = 128, 512, 512, 512
NCORES = 8
BC = B // NCORES          # batch rows per core = 16
KT = D // 128             # 4 contraction tiles
MT = U // 128             # 4 output tiles
CHUNK = 32                # phase-1 time-chunk; N = CHUNK*BC = 512 per matmul
GW = MT * BC              # scan tile width = 64

SCALE = 128.0             # fp8e3m4 weight pre-scale
INV = 1.0 / SCALE

BF16 = mybir.dt.bfloat16
F8E3 = mybir.dt.float8e3
F32 = mybir.dt.float32
NPBF16 = ml_dtypes.bfloat16
NPF8 = ml_dtypes.float8_e3m4
AF = mybir.ActivationFunctionType
ALU = mybir.AluOpType

_CACHE = {}
LAST_RESULTS = None  # test harness reads exec_time_ns / profile from here


def _build(t_steps: int):
    nc = bacc.Bacc("TRN2", target_bir_lowering=False, debug=False)
    nchunk = (t_steps + CHUNK - 1) // CHUNK

    x_d = nc.dram_tensor("xT", [KT, 128, T * BC], BF16, kind="ExternalInput")
    wf_d = nc.dram_tensor("WfT", [128, KT * U], BF16, kind="ExternalInput")
    wh_d = nc.dram_tensor("WhT", [128, KT * U], BF16, kind="ExternalInput")
    uf_d = nc.dram_tensor("UfT", [128, KT * U], F8E3, kind="ExternalInput")
    uh_d = nc.dram_tensor("UhT", [128, KT * U], F8E3, kind="ExternalInput")
    bf_d = nc.dram_tensor("bfT", [128, MT], F32, kind="ExternalInput")
    bh_d = nc.dram_tensor("bhT", [128, MT], F32, kind="ExternalInput")
    eye_d = nc.dram_tensor("eye", [128, 128], F8E3, kind="ExternalInput")
    out_d = nc.dram_tensor("hT_out", [128, KT * BC], F32, kind="ExternalOutput")

    with tile.TileContext(nc) as tc:
        with (
            tc.tile_pool(name="const", bufs=1) as cpool,
            tc.tile_pool(name="xchunk", bufs=3) as xpool,
            tc.tile_pool(name="proj", bufs=16) as projpool,
            tc.tile_pool(name="work", bufs=4) as wpool,
            tc.tile_pool(name="spsum", bufs=4, space="PSUM") as spsum,
            tc.tile_pool(name="ppsum", bufs=2, space="PSUM") as ppsum,
        ):
            # ---- resident tensors ----
            wf_sb = cpool.tile([128, KT * U], BF16, tag="wf")
            wh_sb = cpool.tile([128, KT * U], BF16, tag="wh")
            uf_sb = cpool.tile([128, KT * U], F8E3, tag="uf")
            uh_sb = cpool.tile([128, KT * U], F8E3, tag="uh")
            bf_sb = cpool.tile([128, MT], F32, tag="bf")
            bh_sb = cpool.tile([128, MT], F32, tag="bh")
            eye_sb = cpool.tile([128, 128], F8E3, tag="eye")

            nc.sync.dma_start(wf_sb[:], wf_d[:])
            nc.sync.dma_start(wh_sb[:], wh_d[:])
            nc.sync.dma_start(uf_sb[:], uf_d[:])
            nc.sync.dma_start(uh_sb[:], uh_d[:])
            nc.sync.dma_start(bf_sb[:], bf_d[:])
            nc.sync.dma_start(bh_sb[:], bh_d[:])
            nc.sync.dma_start(eye_sb[:], eye_d[:])

            # per-chunk projection tiles (bf16): free = (t_local, m, b)
            xf_c = [None] * nchunk
            xh_c = [None] * nchunk
            xc_c = [None] * nchunk

            def emit_chunk_dma(c):
                xc = xpool.tile([128, KT * CHUNK * BC], BF16, tag="xc")
                for k in range(KT):
                    nc.sync.dma_start(
                        xc[:, k * CHUNK * BC:(k + 1) * CHUNK * BC],
                        x_d[k, :, c * CHUNK * BC:(c + 1) * CHUNK * BC],
                    )
                xc_c[c] = xc
                xf_c[c] = projpool.tile([128, CHUNK * GW], BF16, tag="xfc", name=f"xfc{c}")
                xh_c[c] = projpool.tile([128, CHUNK * GW], BF16, tag="xhc", name=f"xhc{c}")

            def proj_group_items(c, gi):
                """One (gate, m) projection group of chunk c as a list of
                closures: 4 matmuls + ACT copy, to be fed into scan PE gaps."""
                gate, m = divmod(gi, MT)
                w_sb, b_sb, dst = ((wf_sb, bf_sb, xf_c[c]), (wh_sb, bh_sb, xh_c[c]))[gate]
                xc = xc_c[c]
                state = {}

                def mk_mm(k):
                    def emit():
                        if k == 0:
                            state["ps"] = ppsum.tile(
                                [128, CHUNK * BC], F32, tag="pp", name=f"pp{c}_{gi}"
                            )
                        nc.tensor.matmul(
                            state["ps"][:],
                            w_sb[:, k * U + m * 128: k * U + (m + 1) * 128],
                            xc[:, k * CHUNK * BC:(k + 1) * CHUNK * BC],
                            start=(k == 0), stop=(k == KT - 1),
                            skip_group_check=True,
                        )
                    return emit

                def mk_act():
                    def emit():
                        dv = dst[:].rearrange("p (t m b) -> p t m b", t=CHUNK, m=MT, b=BC)
                        nc.scalar.activation(
                            dv[:, :, m, :],
                            state["ps"][:].rearrange("p (t b) -> p t b", t=CHUNK, b=BC),
                            AF.Identity,
                            bias=b_sb[:, m:m + 1],
                        )
                    return emit

                return [mk_mm(k) for k in range(KT)] + [mk_act()]

            proj_q = []

            def pop_proj(n):
                for _ in range(n):
                    if proj_q:
                        proj_q.pop(0)()

            # prologue: first two chunks fully
            for c in range(min(2, nchunk)):
                emit_chunk_dma(c)
                for gi in range(2 * MT):
                    for item in proj_group_items(c, gi):
                        item()

            # ---- the sequential scan, with projection work interleaved ----
            h = wpool.tile([128, GW], BF16, tag="h")
            nc.vector.memset(h[:], 0.0)

            def gate_accum(z, u_sb, rhs, stop):
                for m in range(MT):
                    for k in range(KT):
                        nc.tensor.matmul(
                            z[:, m * BC:(m + 1) * BC],
                            u_sb[:, k * U + m * 128: k * U + (m + 1) * 128],
                            rhs[:, k * BC:(k + 1) * BC],
                            start=False,
                            stop=(stop and m == MT - 1 and k == KT - 1),
                            skip_group_check=True,
                        )

            def seed(z, xsrc):
                nc.tensor.matmul(z[:], eye_sb[:], xsrc, start=True, stop=False,
                                 skip_group_check=True)

            # z tiles are allocated full-bank ([128, 512] fp32) so the 4-deep
            # rotation maps to 4 distinct PSUM banks: the WAR on a bank then
            # trails by 2 steps and the seed matmuls can run inside the
            # ACT/DVE gaps instead of stalling on the current sigmoid/tanh.
            ZB = 512

            # Dual accumulation: h'(t) = t2(t) + t3(t) with t2 = h-g (mid-step,
            # GpSimd) and t3 = f*S (post-tanh). By linearity
            #   zf(t+1) = seed(xf) + Uf^T t2(t) + Uf^T t3(t)
            # so the t2 half of next step's gate-f matmuls issues during the
            # zh/tanh window, and only the t3 half trails the tanh -> the
            # hn add leaves the critical path (h' materializes lazily for the
            # next step's elementwise ops while those matmuls run).

            # pre-seed zf for step 0 (h0 = 0, so the seed alone is exact)
            zf = spsum.tile([128, ZB], F32, tag="z")
            seed(zf[:, 0:GW], xf_c[0][:, 0:GW])
            t3_prev = None

            for t in range(t_steps):
                c, tl = divmod(t, CHUNK)
                # interleave next-next chunk's projection work into PE gaps
                nxt = c + 2
                if nxt < nchunk:
                    if tl == 0:
                        emit_chunk_dma(nxt)
                        for gi in range(2 * MT):
                            proj_q.extend(proj_group_items(nxt, gi))

                # trailing (on-chain) half of this step's gate-f matmuls
                if t3_prev is not None:
                    gate_accum(zf[:, 0:GW], uf_sb, t3_prev, stop=True)

                # seed zh now: PE fills the sigmoid/mult gap with it
                zh = spsum.tile([128, ZB], F32, tag="z")
                seed(zh[:, 0:GW], xh_c[c][:, tl * GW:(tl + 1) * GW])
                pop_proj(1)

                f = wpool.tile([128, GW], BF16, tag="f")
                nc.scalar.activation(f[:], zf[:, 0:GW], AF.Sigmoid, scale=INV)
                g = wpool.tile([128, GW], BF16, tag="g")
                nc.vector.tensor_tensor(g[:], f[:], h[:], ALU.mult)
                t2 = wpool.tile([128, GW], BF16, tag="t2")
                nc.gpsimd.tensor_tensor(t2[:], h[:], g[:], ALU.subtract)

                gate_accum(zh[:, 0:GW], uh_sb, g, stop=True)

                # next step's zf: seed + the t2 half of its gate-f matmuls,
                # all inside the tanh/update window
                if t + 1 < t_steps:
                    c1, tl1 = divmod(t + 1, CHUNK)
                    zf = spsum.tile([128, ZB], F32, tag="z")
                    seed(zf[:, 0:GW], xf_c[c1][:, tl1 * GW:(tl1 + 1) * GW])
                    gate_accum(zf[:, 0:GW], uf_sb, t2, stop=False)
                pop_proj(1)

                s = wpool.tile([128, GW], BF16, tag="s")
                nc.scalar.activation(s[:], zh[:, 0:GW], AF.Tanh, scale=INV)

                # h' = t2 + f*S   (off the critical path now)
                t3 = wpool.tile([128, GW], BF16, tag="t3")
                nc.vector.tensor_tensor(t3[:], f[:], s[:], ALU.mult)
                last = (t == t_steps - 1)
                hn = wpool.tile([128, GW], F32 if last else BF16, tag="hout" if last else "h")
                nc.vector.tensor_tensor(hn[:], t2[:], t3[:], ALU.add)
                h = hn
                t3_prev = t3

            nc.sync.dma_start(out_d[:], h[:])

    nc.compile()
    return nc


def _prep_weight_t(w, dtype):
    # [D, U] fp32 -> [128, KT*U] with [:, k*U+m] = w[k*128+p, m]
    return np.ascontiguousarray(
        w.reshape(KT, 128, U).transpose(1, 0, 2).reshape(128, KT * U)
    ).astype(dtype)


def kernel(x, Wf, Uf, bf, Wh, Uh, bh):
    global LAST_RESULTS
    x = np.asarray(x, dtype=np.float32)
    Wf = np.asarray(Wf, dtype=np.float32)
    Uf = np.asarray(Uf, dtype=np.float32)
    Wh = np.asarray(Wh, dtype=np.float32)
    Uh = np.asarray(Uh, dtype=np.float32)
    bf = np.asarray(bf, dtype=np.float32)
    bh = np.asarray(bh, dtype=np.float32)

    t_steps = int(os.environ.get("BASS_MGU_T", T))
    if t_steps not in _CACHE:
        _CACHE[t_steps] = _build(t_steps)
    nc = _CACHE[t_steps]

    wf_t = _prep_weight_t(Wf * SCALE, NPBF16)
    wh_t = _prep_weight_t(Wh * SCALE, NPBF16)
    uf_t = _prep_weight_t(np.clip(Uf * SCALE, -15.5, 15.5), NPF8)
    uh_t = _prep_weight_t(np.clip(Uh * SCALE, -15.5, 15.5), NPF8)
    bf_t = np.ascontiguousarray((bf * SCALE).reshape(MT, 128).T).astype(np.float32)
    bh_t = np.ascontiguousarray((bh * SCALE).reshape(MT, 128).T).astype(np.float32)
    eye = np.eye(128, dtype=np.float32).astype(NPF8)

    in_maps = []
    for ci in range(NCORES):
        xc = x[ci * BC:(ci + 1) * BC]                       # [BC, T, D]
        xt = xc.transpose(2, 1, 0)                          # [D, T, BC]
        xt = np.ascontiguousarray(xt.reshape(KT, 128, T * BC)).astype(NPBF16)
        in_maps.append({
            "xT": xt, "WfT": wf_t, "WhT": wh_t, "UfT": uf_t, "UhT": uh_t,
            "bfT": bf_t, "bhT": bh_t, "eye": eye,
        })

    trace = bool(int(os.environ.get("BASS_MGU_TRACE", "0")))
    kw = {}
    if trace and os.environ.get("BASS_TRACE_DIR"):
        kw["tmpdir"] = os.environ["BASS_TRACE_DIR"]
    res = run_bass_kernel_spmd(nc, in_maps, list(range(NCORES)), trace=trace, **kw)
    LAST_RESULTS = res

    out = np.empty((B, U), dtype=np.float32)
    for ci in range(NCORES):
        ho = np.asarray(res.results[ci]["hT_out"])          # [128, KT*BC]
        out[ci * BC:(ci + 1) * BC] = (
            ho.reshape(128, KT, BC).transpose(2, 1, 0).reshape(BC, U)
        )
    return out
